# revision 21
# baseline (speedup 1.0000x reference)
"""Trainium2 Bass kernel for a transformer decoder block (self-attn + cross-attn + MLP).

Sharding: 8 cores = 2 batch groups x 4 cores. Within a group, core c owns
rows r = c (mod 4) of its batch (strided rows balance causal attention work).
K/V are computed replicated within a group (no collectives).

Precision: fp8(e4m3) DoubleRow matmuls for the dense gemms. Weights are
host-split W ~ (W0 + W1)/64 with both parts fp8 at the same scale so the
passes accumulate into one PSUM tile (bf16-level accuracy). LayerNormed
activations are likewise split on-device (h0 + h1, fp8 each); those gemms
run 3 passes (h0W0, h0W1, h1W0). y and the normalized cross-attn output are
single fp8 (measured < 1e-3 rel err contribution). q/k/logits/probs/V stay
bf16; A@V runs probs-stationary so o comes out row-major (no output
transposes); the softmax denominator rides as a ones column in V. The causal
boundary mask is added ON THE PE (identity-stationary matmul accumulating
into the logits psum). Stage-B K2 is built early (off yT8) to fill the PE
while the vector engines run ln1 over all rows.
"""

import os
import sys

for _p in ("/opt/trn_rl_repo", "/root/.axon_site/_ro/trn_rl_repo"):
    if os.path.isdir(_p) and _p not in sys.path:
        sys.path.insert(0, _p)

import numpy as np

B, N, C, H, Y_DIM, HID = 2, 2048, 1024, 16, 1024, 4096
HD = C // H
SCALE = HD ** -0.5
EPS = 1e-5

G = 2          # batch groups
CPG = 4        # cores per group
R = N // CPG   # rows per core (512)
RT = R // 128  # row tiles per core (4)
KB = N // 128  # 128-wide key blocks (16)
NP = KB // 2   # 256-wide key pairs (8)
KIN = C // 128  # contraction tiles for C (8)
WSC = 64.0     # weight fp8 scale
ESC = SCALE / (WSC * WSC)  # logits descale folded into exp()

_CACHE = {}


def _build(mode, skip_gb, skip_bias):
    """mode: 'causal' | 'none' | 'dense'"""
    import concourse.bass as bass
    import concourse.mybir as mybir
    import concourse.tile as tile
    from concourse import bacc
    from concourse.masks import make_identity

    dt = mybir.dt
    F32, BF16, F8 = dt.float32, dt.bfloat16, dt.float8e4
    AF = mybir.ActivationFunctionType
    ALU = mybir.AluOpType
    DR = mybir.MatmulPerfMode.DoubleRow

    nc = bacc.Bacc("TRN2", target_bir_lowering=False, debug=False, num_devices=8)

    # ---- DRAM I/O ----------------------------------------------------------
    def din(name, shape, dtype=F32):
        return nc.dram_tensor(name, list(shape), dtype,
                              kind="ExternalInput").ap()

    x_my = din("x_my", (R, C), BF16)
    x_full = din("x_full", (N, C), BF16)
    yT8 = din("yT8", (Y_DIM, N), F8)
    w_qk = [din(f"w_qk{i}", (C, 2 * C), F8) for i in range(2)]
    w_v = [din(f"w_v{i}", (C, C), F8) for i in range(2)]
    w_pj = [din(f"w_pj{i}", (C, C), F8) for i in range(2)]
    w_q2 = [din(f"w_q2{i}", (C, C), F8) for i in range(2)]
    w_kv = [din(f"w_kv{i}", (Y_DIM, 2 * C), F8) for i in range(2)]
    w_p2 = [din(f"w_p2{i}", (C, C), F8) for i in range(2)]
    w_f1 = [din(f"w_f1{i}", (C, HID), F8) for i in range(2)]
    w_f2 = [din(f"w_f2{i}", (HID, C), F8) for i in range(2)]
    if mode == "causal":
        bmask = din("bmask", (128, 2, 64), BF16)
    if mode == "dense":
        maskT = din("maskT", (N, R), BF16)
    if not skip_bias:
        projb = din("projb", (C,))
        proj2b = din("proj2b", (C,))
        fc1b = din("fc1b", (HID,))
        fc2b = din("fc2b", (C,))
    if not skip_gb:
        lng = {k: din("g_" + k, (HID if k == "mln2" else C,))
               for k in ("ln1", "aln2", "a2ln", "mln1", "mln2")}
        lnb = {k: din("b_" + k, (HID if k == "mln2" else C,))
               for k in ("ln1", "aln2", "a2ln", "mln1", "mln2")}
    out_my = nc.dram_tensor("out_my", [R, C], F32, kind="ExternalOutput").ap()

    def bcast(vec_ap, n):
        return bass.AP(tensor=vec_ap.tensor, offset=vec_ap.offset,
                       ap=[[0, 128]] + vec_ap.ap)

    def wtiles(w_ap):
        # DRAM [K, M] -> [128, K//128, M] view for kb-chunked tile DMA
        return w_ap.rearrange("(kb p) m -> p kb m", p=128)

    with tile.TileContext(nc) as tc:
        with tc.tile_pool(name="singles", bufs=1) as singles, \
             tc.tile_pool(name="stats", bufs=3) as stats, \
             tc.tile_pool(name="resid", bufs=1) as resid:

            ident8 = singles.tile([128, 128], F8, name="ident8", tag="ident8")
            make_identity(nc, ident8)
            identb = singles.tile([128, 128], BF16, name="identb", tag="identb")
            make_identity(nc, identb)
            eps_t = singles.tile([128, 1], F32, name="eps", tag="eps")
            nc.vector.memset(eps_t, EPS)


            if mode == "causal":
                bmask_t = singles.tile([128, 2, 64], BF16, name="bmask", tag="bmask")
                nc.sync.dma_start(out=bmask_t, in_=bmask)
            maskT_t = None
            if mode == "dense":
                maskT_t = [singles.tile([128, R], BF16, name=f"maskT{j}", tag=f"maskT{j}")
                           for j in range(KB)]
                for j in range(KB):
                    nc.sync.dma_start(out=maskT_t[j], in_=maskT[j * 128:(j + 1) * 128, :])

            gb_tiles = {}
            if not skip_gb:
                for k in ("ln1", "aln2", "a2ln", "mln1", "mln2"):
                    d = HID if k == "mln2" else C
                    gt = singles.tile([128, d], F32, name=f"g_{k}", tag=f"g_{k}")
                    bt = singles.tile([128, d], F32, name=f"b_{k}", tag=f"b_{k}")
                    nc.sync.dma_start(out=gt, in_=bcast(lng[k], d))
                    nc.sync.dma_start(out=bt, in_=bcast(lnb[k], d))
                    gb_tiles[k] = (gt, bt)

            # ---- helpers ---------------------------------------------------
            def ln_fp8_split(wp, x_in, d, key, out0, out1=None):
                """LayerNorm of x_in [128, d] -> fp8 out0 + same-scale fp8
                residual out1 (split on GPSIMD, stats/apply on DVE)."""
                nsub = max(1, d // 512)
                st = stats.tile([128, nsub, 6], F32, name="bnst", tag="bnst")
                xr = x_in.rearrange("p (s q) -> p s q", s=nsub)
                for s in range(nsub):
                    nc.vector.bn_stats(out=st[:, s, :], in_=xr[:, s, :])
                mv = stats.tile([128, 2], F32, name="bnmv", tag="bnmv")
                nc.vector.bn_aggr(out=mv, in_=st)
                sd = stats.tile([128, 1], F32, name="bnsd", tag="bnsd")
                nc.scalar.activation(out=sd, in_=mv[:, 1:2], func=AF.Sqrt, bias=eps_t)
                ri = stats.tile([128, 1], F32, name="bnri", tag="bnri")
                nc.vector.reciprocal(out=ri, in_=sd)
                nm = stats.tile([128, 1], F32, name="bnnm", tag="bnnm")
                nc.vector.tensor_scalar(out=nm, in0=mv[:, 0:1], scalar1=ri,
                                        scalar2=-1.0, op0=ALU.mult, op1=ALU.mult)
                hf = wp.tile([128, d], BF16, name="lnf", tag="lnf")
                nc.vector.tensor_scalar(out=hf, in0=x_in, scalar1=ri,
                                        scalar2=nm, op0=ALU.mult, op1=ALU.add)
                if not skip_gb:
                    gt, bt = gb_tiles[key]
                    nc.vector.tensor_tensor(out=hf, in0=hf, in1=gt[:, :d], op=ALU.mult)
                    nc.vector.tensor_tensor(out=hf, in0=hf, in1=bt[:, :d], op=ALU.add)
                # fp8 narrowing copy on Act (idle during ln phases); residual
                # subtract stays on Pool so DVE keeps only stats+apply.
                nc.scalar.copy(out=out0, in_=hf)
                if out1 is not None:
                    nc.gpsimd.tensor_tensor(out=out1, in0=hf, in1=out0,
                                            op=ALU.subtract)

            def psum_copy(i, out, in_):
                if i % 2 == 0:
                    nc.vector.tensor_copy(out=out, in_=in_)
                else:
                    nc.scalar.copy(out=out, in_=in_)

            def psum_scale(i, out, in_, s):
                if i % 2 == 0:
                    nc.vector.tensor_scalar(out=out, in0=in_, scalar1=s,
                                            scalar2=None, op0=ALU.mult)
                else:
                    nc.scalar.mul(out=out, in_=in_, mul=s)

            def transpose_f8(pp, dst, src_tile, rt_idx, nblk, eng):
                """fp8 src [128, nblk*128] -> dst [128, nblk, *] cols rt_idx*128."""
                for g in range(0, nblk, 8):
                    nb = min(8, nblk - g)
                    # fp8 PE transpose writes with an element step of 2
                    ptb = pp.tile([128, 8, 128, 2], F8, name="tpb", tag="tpb")
                    for k in range(nb):
                        nc.tensor.transpose(
                            ptb[:, k, :, 0],
                            src_tile[:, (g + k) * 128:(g + k + 1) * 128], ident8)
                    if eng is nc.vector:
                        nc.vector.tensor_copy(
                            out=dst[:, g:g + nb, rt_idx * 128:(rt_idx + 1) * 128],
                            in_=ptb[:, 0:nb, :, 0])
                    else:
                        nc.scalar.copy(
                            out=dst[:, g:g + nb, rt_idx * 128:(rt_idx + 1) * 128],
                            in_=ptb[:, 0:nb, :, 0])

            def dr_gemm(ps, passes, kp, asl, wsl):
                """Accumulate fp8 DoubleRow passes into psum ps.
                passes: list of (act_tile, w_tile); asl/wsl: (tile, k) -> AP."""
                npass = len(passes)
                for pi, (at, wt) in enumerate(passes):
                    for k in range(kp):
                        nc.tensor.matmul(
                            ps, wsl(wt, k), asl(at, k),
                            start=(pi == 0 and k == 0),
                            stop=(pi == npass - 1 and k == kp - 1),
                            perf_mode=DR)

            def dr_gemm_rows(ps, passes, kp, asl, wsl):
                """Row-major out: stationary = transposed acts, moving = W."""
                npass = len(passes)
                for pi, (at, wt) in enumerate(passes):
                    for k in range(kp):
                        nc.tensor.matmul(
                            ps, asl(at, k), wsl(wt, k),
                            start=(pi == 0 and k == 0),
                            stop=(pi == npass - 1 and k == kp - 1),
                            perf_mode=DR)

            # ================================================================
            # Stage A
            # ================================================================
            x_my_t = [None] * RT
            x1_my = [None] * RT
            x2_my = [None] * RT

            def attention(kT_t, qT_t, vtiles, causal, consume_h, h_lo=0, h_hi=H,
                          pools=None, filler=None, dr=False):
                """kT_t: 2 tiles [128, KIN//2, N]; vtiles: [half][jp] views
                [128, 2, 8, 65]. consume_h(h, o_ps): col 64 = denom.
                filler(h): emits independent PE work after each head.
                dr: fp8 DoubleRow A@V (probs fp8; vtiles[jp] = [128,2,H,65]
                fp8, key pair jj packed as the DR plane)."""
                ctx_pools = pools
                if ctx_pools is None:
                    import contextlib
                    stack = contextlib.ExitStack()
                    pp = stack.enter_context(tc.tile_pool(name="atp", bufs=3, space="PSUM"))
                    po = stack.enter_context(tc.tile_pool(name="ato", bufs=2, space="PSUM"))
                    aw = stack.enter_context(tc.tile_pool(name="atw", bufs=3))
                else:
                    stack = None
                    pp, po, aw = ctx_pools
                if True:
                    for h in range(h_lo, h_hi):
                        kT_h = kT_t[h // 8]
                        hp, hh = (h // 2) % 4, h % 2
                        o_ps = po.tile([128, RT, 128], F32, name="ops", tag="ops")
                        for jp in range(NP):
                            r0 = 64 * jp if causal else 0
                            nj = R - r0
                            lg = pp.tile([128, 2, 512], F32, name="lg", tag="lg")
                            for jj in range(2):
                                J = 2 * jp + jj
                                nc.tensor.matmul(
                                    lg[:, jj, 0:nj],
                                    kT_h[hh * 64:hh * 64 + 64, hp, J * 128:(J + 1) * 128],
                                    qT_t[hh * 64:hh * 64 + 64, (h // 2), r0:R],
                                    start=True, stop=not causal,
                                    skip_group_check=causal)
                                if causal:
                                    nc.tensor.matmul(
                                        lg[:, jj, 0:64], identb,
                                        bmask_t[:, jj, :],
                                        start=False, stop=True,
                                        skip_group_check=True)
                            if mode == "dense":
                                for jj in range(2):
                                    nc.vector.tensor_tensor(
                                        out=lg[:, jj, 0:nj], in0=lg[:, jj, 0:nj],
                                        in1=maskT_t[2 * jp + jj][:, r0:R], op=ALU.add)
                            pt = aw.tile([128, 2, 512], F8 if dr else BF16,
                                         name="pt", tag="pt")
                            nc.scalar.activation(out=pt[:, :, 0:nj], in_=lg[:, :, 0:nj],
                                                 func=AF.Exp, scale=ESC)
                            rc_lo = jp // 2 if causal else 0
                            if dr:
                                for rc in range(RT):
                                    soff = rc * 128
                                    nc.tensor.matmul(
                                        o_ps[:, rc, 0:65],
                                        pt[:, 0:2, soff:soff + 128],
                                        vtiles[jp][:, 0:2, h, 0:65],
                                        start=(jp == 0 and rc == 0),
                                        stop=(jp == NP - 1),
                                        skip_group_check=True,
                                        perf_mode=DR)
                            else:
                                for rc in range(rc_lo, RT):
                                    soff = rc * 128 - r0
                                    for jj in range(2):
                                        last = (jp == 2 * rc + 1 and jj == 1) if causal \
                                            else (jp == NP - 1 and jj == 1)
                                        mv = vtiles[h // 8][jp][:, jj, h % 8, :]
                                        if soff < 0:
                                            nc.tensor.matmul(
                                                o_ps[64:128, rc, 0:65],
                                                pt[:, jj, 0:64], mv,
                                                start=False, stop=last,
                                                skip_group_check=True)
                                        else:
                                            nc.tensor.matmul(
                                                o_ps[:, rc, 0:65],
                                                pt[:, jj, soff:soff + 128], mv,
                                                start=(jp == 0 and jj == 0 and rc == 0),
                                                stop=last,
                                                skip_group_check=True)
                        consume_h(h, o_ps)
                        if filler is not None:
                            filler(h)
                    if stack is not None:
                        stack.close()

            with tc.tile_pool(name="bkv", bufs=1) as bkv:
                k2T = bkv.tile([128, KIN, N], BF16, name="k2T", tag="k2T")

                # K2 first, in its own scope (PE fills while DVE starts ln1;
                # yT freed before the big stage-A pools open)
                with tc.tile_pool(name="ypool", bufs=1) as yp, \
                     tc.tile_pool(name="wst0", bufs=2) as wst0, \
                     tc.tile_pool(name="psM0", bufs=3, space="PSUM") as psM0:
                    yT_t = yp.tile([128, KIN, N], F8, name="yT", tag="yT")
                    nc.sync.dma_start(out=yT_t, in_=yT8.rearrange("(kb p) n -> p kb n", p=128))
                    for m in range(KIN):
                        w0t = wst0.tile([128, KIN, 128], F8, name="k2w0", tag="wa")
                        nc.sync.dma_start(out=w0t, in_=wtiles(w_kv[0])[:, :, m * 128:(m + 1) * 128])
                        for n in range(N // 512):
                            ps = psM0.tile([128, 512], F32, name="kps", tag="mps")
                            dr_gemm(
                                ps, [(yT_t, w0t)], KIN // 2,
                                lambda a, k, n=n: a[:, 2 * k:2 * k + 2, n * 512:(n + 1) * 512],
                                lambda w, k: w[:, 2 * k:2 * k + 2, :])
                            psum_copy(m + n, k2T[:, m, n * 512:(n + 1) * 512], ps)

                with tc.tile_pool(name="akv", bufs=1) as akv:
                    kT2h = [akv.tile([128, KIN // 2, N], BF16, name=f"kTh{i}", tag=f"kTh{i}")
                            for i in range(2)]
                    qT = akv.tile([128, KIN, R], BF16, name="qT", tag="qT")
                    v_2h = [[akv.tile([128, 2, 8, 65], BF16, name=f"v_{i}_{j}", tag=f"v_{i}_{j}")
                             for j in range(NP)] for i in range(2)]

                    with tc.tile_pool(name="osbp", bufs=1) as osbp:
                        osb = [osbp.tile([128, C], BF16, name=f"osb{t}", tag=f"osb{t}")
                               for t in range(RT)]

                        def consume_A(h, o_ps):
                            rd = stats.tile([128, RT, 1], F32, name="rd", tag="rd")
                            nc.vector.reciprocal(out=rd, in_=o_ps[:, :, 64:65])
                            for rc in range(RT):
                                nc.vector.tensor_scalar(
                                    out=osb[rc][:, h * 64:(h + 1) * 64],
                                    in0=o_ps[:, rc, 0:64], scalar1=rd[:, rc, :],
                                    scalar2=None, op0=ALU.mult)

                        with tc.tile_pool(name="abuild", bufs=1) as ab, \
                             tc.tile_pool(name="wst", bufs=2) as wst, \
                             tc.tile_pool(name="wstv", bufs=1) as wstv:
                            h0T = ab.tile([128, KIN, N], F8, name="h0T", tag="h0T")
                            h1T = ab.tile([128, KIN, N], F8, name="h1T", tag="h1T")

                            with tc.tile_pool(name="awork", bufs=4) as awk, \
                                 tc.tile_pool(name="apsT", bufs=2, space="PSUM") as apsT, \
                                 tc.tile_pool(name="apsQ", bufs=3, space="PSUM") as apsQ:
                                # full-row ln1 -> h0/h1 fp8 -> transpose to h0T/h1T
                                for t in range(KB):
                                    xf = awk.tile([128, C], BF16, name="xfull", tag="xfull")
                                    nc.sync.dma_start(out=xf, in_=x_full[t * 128:(t + 1) * 128, :])
                                    h0 = awk.tile([128, C], F8, name="h0", tag="h0")
                                    h1 = awk.tile([128, C], F8, name="h1", tag="h1")
                                    ln_fp8_split(awk, xf, C, "ln1", h0, h1)
                                    transpose_f8(apsT, h0T, h0, t, KIN,
                                                 nc.vector if t % 2 == 0 else nc.scalar)
                                    transpose_f8(apsT, h1T, h1, t, KIN,
                                                 nc.scalar if t % 2 == 0 else nc.vector)

                                # own-rows ln1 (x_my) -> hmyT0 -> qT (2-pass)
                                hmyT0 = ab.tile([128, KIN, R], F8, name="hmyT0",
                                                tag="hmyT0")
                                for t in range(RT):
                                    x_my_t[t] = resid.tile([128, C], BF16, name=f"xmy{t}", tag=f"xmy{t}")
                                    nc.sync.dma_start(out=x_my_t[t], in_=x_my[t * 128:(t + 1) * 128, :])
                                    hm0 = awk.tile([128, C], F8, name="hm0", tag="h0")
                                    ln_fp8_split(awk, x_my_t[t], C, "ln1", hm0)
                                    transpose_f8(apsT, hmyT0, hm0, t, KIN,
                                                 nc.vector if t % 2 == 0 else nc.scalar)
                                for m in range(KIN):
                                    w0t = wst.tile([128, KIN, 128], F8, name="qw0", tag="wa")
                                    w1t = wst.tile([128, KIN, 128], F8, name="qw1", tag="wb")
                                    nc.sync.dma_start(out=w0t, in_=wtiles(w_qk[0])[:, :, m * 128:(m + 1) * 128])
                                    nc.sync.dma_start(out=w1t, in_=wtiles(w_qk[1])[:, :, m * 128:(m + 1) * 128])
                                    ps = apsQ.tile([128, R], F32, name="qps", tag="mps")
                                    dr_gemm(
                                        ps, [(hmyT0, w0t), (hmyT0, w1t)], KIN // 2,
                                        lambda a, k: a[:, 2 * k:2 * k + 2, :],
                                        lambda w, k: w[:, 2 * k:2 * k + 2, :])
                                    nc.vector.tensor_copy(out=qT[:, m, :], in_=ps)

                            # ---- interleaved K/V builds and attention ------
                            with tc.tile_pool(name="apsM", bufs=2, space="PSUM") as apsM, \
                                 tc.tile_pool(name="atp", bufs=2, space="PSUM") as atp, \
                                 tc.tile_pool(name="ato", bufs=2, space="PSUM") as ato, \
                                 tc.tile_pool(name="atw", bufs=3) as atw:

                                def kv_units(mh):
                                    """List of thunks: kT m-units, v weight+pair units,
                                    ones-col memsets. Each emits independent PE work."""
                                    units = []

                                    def k_unit(m):
                                        w0t = wst.tile([128, KIN, 128], F8, name="kw0", tag="wa")
                                        w1t = wst.tile([128, KIN, 128], F8, name="kw1", tag="wb")
                                        c0 = C + m * 128
                                        nc.sync.dma_start(out=w0t, in_=wtiles(w_qk[0])[:, :, c0:c0 + 128])
                                        nc.sync.dma_start(out=w1t, in_=wtiles(w_qk[1])[:, :, c0:c0 + 128])
                                        for n in range(N // 512):
                                            ps = apsM.tile([128, 512], F32, name="kps", tag="mps")
                                            dr_gemm(
                                                ps, [(h0T, w0t), (h0T, w1t)], KIN // 2,
                                                lambda a, k, n=n: a[:, 2 * k:2 * k + 2, n * 512:(n + 1) * 512],
                                                lambda w, k: w[:, 2 * k:2 * k + 2, :])
                                            nc.vector.tensor_copy(
                                                out=kT2h[mh][:, m % 4, n * 512:(n + 1) * 512], in_=ps)

                                    for m in range(4 * mh, 4 * mh + 4):
                                        units.append(lambda m=m: k_unit(m))

                                    vw = []

                                    def v_wdma(half):
                                        w0t = wstv.tile([128, KIN, 512], F8, name="vw0", tag="vw0")
                                        w1t = wstv.tile([128, KIN, 512], F8, name="vw1", tag="vw1")
                                        c0 = half * 512
                                        nc.sync.dma_start(out=w0t, in_=wtiles(w_v[0])[:, :, c0:c0 + 512])
                                        nc.sync.dma_start(out=w1t, in_=wtiles(w_v[1])[:, :, c0:c0 + 512])
                                        vw.extend([w0t, w1t])

                                    def v_unit(half, t0):
                                        for t in (t0, t0 + 1):
                                            ps = apsM.tile([128, 512], F32, name="vps", tag="mps")
                                            dr_gemm_rows(
                                                ps, [(h0T, vw[0]), (h0T, vw[1]), (h1T, vw[0])], KIN // 2,
                                                lambda a, k, t=t: a[:, 2 * k:2 * k + 2, t * 128:(t + 1) * 128],
                                                lambda w, k: w[:, 2 * k:2 * k + 2, :])
                                            dst = v_2h[half][t // 2][:, t % 2, :, 0:64]
                                            nc.vector.tensor_scalar(
                                                out=dst, in0=ps.rearrange("p (h c) -> p h c", c=64),
                                                scalar1=1.0 / WSC, scalar2=None, op0=ALU.mult)

                                    units.append(lambda: v_wdma(mh))
                                    for t0 in range(0, KB, 2):
                                        units.append(lambda t0=t0: v_unit(mh, t0))

                                    def ones(half):
                                        for j in range(NP):
                                            nc.gpsimd.memset(v_2h[half][j][:, :, :, 64:65], 1.0)

                                    units.append(lambda: ones(mh))
                                    return units

                                for u in kv_units(0):
                                    u()
                                units1 = kv_units(1)

                                def filler(h):
                                    for _ in range(2):
                                        if units1:
                                            units1.pop(0)()

                                attention(kT2h, qT, v_2h, causal=(mode == "causal"),
                                          consume_h=consume_A, h_lo=0, h_hi=H // 2,
                                          pools=(atp, ato, atw), filler=filler)
                                while units1:
                                    units1.pop(0)()
                                attention(kT2h, qT, v_2h, causal=(mode == "causal"),
                                          consume_h=consume_A, h_lo=H // 2, h_hi=H,
                                          pools=(atp, ato, atw))

                        with tc.tile_pool(name="apj", bufs=1) as apj, \
                             tc.tile_pool(name="apw", bufs=4) as apw, \
                             tc.tile_pool(name="apwt", bufs=2) as apwt, \
                             tc.tile_pool(name="apps", bufs=2, space="PSUM") as app, \
                             tc.tile_pool(name="appm", bufs=3, space="PSUM") as appm:
                            olnT0 = apj.tile([128, KIN, R], F8, name="olnT0", tag="olnT0")
                            olnT1 = apj.tile([128, KIN, R], F8, name="olnT1", tag="olnT1")
                            for t in range(RT):
                                o0 = apw.tile([128, C], F8, name="o0", tag="o0")
                                o1 = apw.tile([128, C], F8, name="o1", tag="o1")
                                ln_fp8_split(apw, osb[t], C, "aln2", o0, o1)
                                transpose_f8(app, olnT0, o0, t, KIN, nc.vector)
                                transpose_f8(app, olnT1, o1, t, KIN, nc.scalar)
                            if not skip_bias:
                                pb_t = apj.tile([128, C], F32, name="projb", tag="projb")
                                nc.sync.dma_start(out=pb_t, in_=bcast(projb, C))
                            for t in range(RT):
                                x1_my[t] = resid.tile([128, C], BF16, name=f"x1my{t}", tag=f"x1my{t}")
                            for nch in range(2):
                                w0t = apwt.tile([128, KIN, 512], F8, name="pw0", tag="pw0")
                                w1t = apwt.tile([128, KIN, 512], F8, name="pw1", tag="pw1")
                                nc.sync.dma_start(out=w0t, in_=wtiles(w_pj[0])[:, :, nch * 512:(nch + 1) * 512])
                                nc.sync.dma_start(out=w1t, in_=wtiles(w_pj[1])[:, :, nch * 512:(nch + 1) * 512])
                                for rt in range(RT):
                                    ps = appm.tile([128, 512], F32, name="pps", tag="mps")
                                    dr_gemm_rows(
                                        ps, [(olnT0, w0t), (olnT0, w1t), (olnT1, w0t)], KIN // 2,
                                        lambda a, k, rt=rt: a[:, 2 * k:2 * k + 2, rt * 128:(rt + 1) * 128],
                                        lambda w, k: w[:, 2 * k:2 * k + 2, :])
                                    sl = slice(nch * 512, (nch + 1) * 512)
                                    nc.vector.scalar_tensor_tensor(
                                        out=x1_my[rt][:, sl], in0=ps, scalar=1.0 / WSC,
                                        in1=x_my_t[rt][:, sl], op0=ALU.mult, op1=ALU.add)
                                    if not skip_bias:
                                        nc.gpsimd.tensor_tensor(out=x1_my[rt][:, sl],
                                                                in0=x1_my[rt][:, sl],
                                                                in1=pb_t[:, sl], op=ALU.add)

                # ============================================================
                # Stage B: cross-attention (k2T ready; build v2, q2 here)
                # ============================================================
                with tc.tile_pool(name="bat", bufs=1) as bat:
                    q2T = bat.tile([128, KIN, R], BF16, name="q2T", tag="q2T")
                    v2_f8 = [bat.tile([128, 2, H, 65], F8, name=f"v2_{j}", tag=f"v2_{j}")
                             for j in range(NP)]
                    o2_8 = [bat.tile([128, C], F8, name=f"o28_{t}", tag=f"o28_{t}")
                            for t in range(RT)]

                    with tc.tile_pool(name="ypool2", bufs=1) as yp2, \
                         tc.tile_pool(name="bq", bufs=1) as bq, \
                         tc.tile_pool(name="bwork", bufs=4) as bwk, \
                         tc.tile_pool(name="bwst", bufs=2) as bwst, \
                         tc.tile_pool(name="bwstv", bufs=1) as bwstv, \
                         tc.tile_pool(name="bpsT", bufs=2, space="PSUM") as bpsT, \
                         tc.tile_pool(name="bpsM", bufs=3, space="PSUM") as bpsM:
                        yT2 = yp2.tile([128, KIN, N], F8, name="yT2", tag="yT2")
                        nc.sync.dma_start(out=yT2, in_=yT8.rearrange("(kb p) n -> p kb n", p=128))
                        # V2 (1-pass off yT8, fp8 out for DR A@V)
                        for half in range(2):
                            w0t = bwstv.tile([128, KIN, 512], F8, name="v2w0", tag="v2w0")
                            c0 = C + half * 512
                            nc.sync.dma_start(out=w0t, in_=wtiles(w_kv[0])[:, :, c0:c0 + 512])
                            for t in range(KB):
                                ps = bpsM.tile([128, 512], F32, name="v2ps", tag="mps")
                                dr_gemm_rows(
                                    ps, [(yT2, w0t)], KIN // 2,
                                    lambda a, k, t=t: a[:, 2 * k:2 * k + 2, t * 128:(t + 1) * 128],
                                    lambda w, k: w[:, 2 * k:2 * k + 2, :])
                                dst = v2_f8[t // 2][:, t % 2, half * 8:(half + 1) * 8, 0:64]
                                psum_scale(0, dst,
                                           ps.rearrange("p (h c) -> p h c", c=64), 1.0 / WSC)
                        for j in range(NP):
                            nc.gpsimd.memset(v2_f8[j][:, :, :, 64:65], 1.0)

                        # h2 = ln(x1) single fp8 -> q2 (1-pass)
                        h2T0 = bq.tile([128, KIN, R], F8, name="h2T0", tag="h2T0")
                        for t in range(RT):
                            h20 = bwk.tile([128, C], F8, name="h20", tag="h20")
                            ln_fp8_split(bwk, x1_my[t], C, "a2ln", h20)
                            transpose_f8(bpsT, h2T0, h20, t, KIN,
                                         nc.vector if t % 2 == 0 else nc.scalar)
                        for m in range(KIN):
                            w0t = bwst.tile([128, KIN, 128], F8, name="q2w0", tag="wa")
                            nc.sync.dma_start(out=w0t, in_=wtiles(w_q2[0])[:, :, m * 128:(m + 1) * 128])
                            ps = bpsM.tile([128, R], F32, name="q2ps", tag="mps")
                            dr_gemm(
                                ps, [(h2T0, w0t)], KIN // 2,
                                lambda a, k: a[:, 2 * k:2 * k + 2, :],
                                lambda w, k: w[:, 2 * k:2 * k + 2, :])
                            nc.vector.tensor_copy(out=q2T[:, m, :], in_=ps)

                    def consume_B(h, o_ps):
                        rd = stats.tile([128, RT, 1], F32, name="rd2", tag="rd2")
                        nc.vector.reciprocal(out=rd, in_=o_ps[:, :, 64:65])
                        for rc in range(RT):
                            nc.vector.tensor_scalar(
                                out=o2_8[rc][:, h * 64:(h + 1) * 64],
                                in0=o_ps[:, rc, 0:64], scalar1=rd[:, rc, :],
                                scalar2=WSC, op0=ALU.mult, op1=ALU.mult)

                    attention([k2T[:, 0:4, :], k2T[:, 4:8, :]], q2T,
                              v2_f8, causal=False, consume_h=consume_B, dr=True)

                    with tc.tile_pool(name="bpj", bufs=1) as bpj, \
                         tc.tile_pool(name="bpwt", bufs=2) as bpwt, \
                         tc.tile_pool(name="bpps", bufs=2, space="PSUM") as bpp, \
                         tc.tile_pool(name="bppm", bufs=3, space="PSUM") as bppm:
                        o2T = bpj.tile([128, KIN, R], F8, name="o2T", tag="o2T")
                        for t in range(RT):
                            transpose_f8(bpp, o2T, o2_8[t], t, KIN, nc.vector)
                        if not skip_bias:
                            p2b_t = bpj.tile([128, C], F32, name="proj2b", tag="proj2b")
                            nc.sync.dma_start(out=p2b_t, in_=bcast(proj2b, C))
                        for t in range(RT):
                            x2_my[t] = resid.tile([128, C], BF16, name=f"x2my{t}", tag=f"xmy{t}")
                        for nch in range(2):
                            w0t = bpwt.tile([128, KIN, 512], F8, name="p2w0", tag="p2w0")
                            nc.sync.dma_start(out=w0t, in_=wtiles(w_p2[0])[:, :, nch * 512:(nch + 1) * 512])
                            for rt in range(RT):
                                ps = bppm.tile([128, 512], F32, name="p2ps", tag="mps")
                                dr_gemm_rows(
                                    ps, [(o2T, w0t)], KIN // 2,
                                    lambda a, k, rt=rt: a[:, 2 * k:2 * k + 2, rt * 128:(rt + 1) * 128],
                                    lambda w, k: w[:, 2 * k:2 * k + 2, :])
                                sl = slice(nch * 512, (nch + 1) * 512)
                                nc.vector.scalar_tensor_tensor(
                                    out=x2_my[rt][:, sl], in0=ps, scalar=1.0 / (WSC * WSC),
                                    in1=x1_my[rt][:, sl], op0=ALU.mult, op1=ALU.add)
                                if not skip_bias:
                                    nc.gpsimd.tensor_tensor(out=x2_my[rt][:, sl],
                                                            in0=x2_my[rt][:, sl],
                                                            in1=p2b_t[:, sl], op=ALU.add)

            # ================================================================
            # Stage C: MLP (all SBUF-resident)
            # ================================================================
            with tc.tile_pool(name="cp", bufs=1) as cp, \
                 tc.tile_pool(name="cw", bufs=3) as cw, \
                 tc.tile_pool(name="cwt", bufs=2) as cwt, \
                 tc.tile_pool(name="cps", bufs=2, space="PSUM") as cps, \
                 tc.tile_pool(name="cpm", bufs=3, space="PSUM") as cpm:
                h3T0 = cp.tile([128, KIN, R], F8, name="h3T0", tag="h3T0")
                h3T1 = cp.tile([128, KIN, R], F8, name="h3T1", tag="h3T1")
                for t in range(RT):
                    h30 = cw.tile([128, C], F8, name="h30", tag="h30")
                    h31 = cw.tile([128, C], F8, name="h31", tag="h31")
                    ln_fp8_split(cw, x2_my[t], C, "mln1", h30, h31)
                    transpose_f8(cps, h3T0, h30, t, KIN, nc.vector)
                    transpose_f8(cps, h3T1, h31, t, KIN, nc.scalar)

                h4_bf = [cp.tile([128, HID], BF16, name=f"h4_{t}", tag=f"h4_{t}")
                         for t in range(RT)]
                if not skip_bias:
                    fb1_t = cp.tile([128, HID], F32, name="fc1b", tag="fc1b")
                    nc.sync.dma_start(out=fb1_t, in_=bcast(fc1b, HID))
                for nch in range(HID // 512):
                    w0t = cwt.tile([128, KIN, 512], F8, name="f1w0", tag="cwa")
                    w1t = cwt.tile([128, KIN, 512], F8, name="f1w1", tag="cwb")
                    nc.sync.dma_start(out=w0t, in_=wtiles(w_f1[0])[:, :, nch * 512:(nch + 1) * 512])
                    nc.sync.dma_start(out=w1t, in_=wtiles(w_f1[1])[:, :, nch * 512:(nch + 1) * 512])
                    for rt in range(RT):
                        ps = cpm.tile([128, 512], F32, name="f1ps", tag="mps")
                        dr_gemm_rows(
                            ps, [(h3T0, w0t), (h3T0, w1t), (h3T1, w0t)], KIN // 2,
                            lambda a, k, rt=rt: a[:, 2 * k:2 * k + 2, rt * 128:(rt + 1) * 128],
                            lambda w, k: w[:, 2 * k:2 * k + 2, :])
                        sl = slice(nch * 512, (nch + 1) * 512)
                        if skip_bias:
                            nc.scalar.activation(out=h4_bf[rt][:, sl], in_=ps,
                                                 func=AF.Gelu, scale=1.0 / WSC)
                        else:
                            nc.vector.tensor_scalar(out=ps, in0=ps, scalar1=1.0 / WSC,
                                                    scalar2=None, op0=ALU.mult)
                            nc.vector.tensor_tensor(out=ps, in0=ps, in1=fb1_t[:, sl],
                                                    op=ALU.add)
                            nc.scalar.activation(out=h4_bf[rt][:, sl], in_=ps,
                                                 func=AF.Gelu)

                h5T0 = cp.tile([128, HID // 128, R], F8, name="h5T0", tag="h5T0")
                h5T1 = cp.tile([128, HID // 128, R], F8, name="h5T1", tag="h5T1")
                for t in range(RT):
                    h50 = cw.tile([128, HID], F8, name="h50", tag="h50", bufs=2)
                    h51 = cw.tile([128, HID], F8, name="h51", tag="h51", bufs=2)
                    ln_fp8_split(cw, h4_bf[t], HID, "mln2", h50, h51)
                    transpose_f8(cps, h5T0, h50, t, HID // 128, nc.vector)
                    transpose_f8(cps, h5T1, h51, t, HID // 128, nc.scalar)

                if not skip_bias:
                    fb2_t = cp.tile([128, C], F32, name="fc2b", tag="fc2b")
                    nc.sync.dma_start(out=fb2_t, in_=bcast(fc2b, C))
                for nch in range(2):
                    w0h = []
                    w1h = []
                    for kh in range(2):
                        w0t = cwt.tile([128, HID // 256, 512], F8, name=f"f2w0_{kh}", tag="cwa")
                        w1t = cwt.tile([128, HID // 256, 512], F8, name=f"f2w1_{kh}", tag="cwb")
                        nc.sync.dma_start(
                            out=w0t, in_=wtiles(w_f2[0])[:, kh * 16:(kh + 1) * 16,
                                                         nch * 512:(nch + 1) * 512])
                        nc.sync.dma_start(
                            out=w1t, in_=wtiles(w_f2[1])[:, kh * 16:(kh + 1) * 16,
                                                         nch * 512:(nch + 1) * 512])
                        w0h.append(w0t)
                        w1h.append(w1t)
                    for rt in range(RT):
                        ps = cpm.tile([128, 512], F32, name="f2ps", tag="mps")
                        dr_gemm_rows(
                            ps, [(h5T0, w0h), (h5T0, w1h), (h5T1, w0h)], HID // 256,
                            lambda a, k, rt=rt: a[:, 2 * k:2 * k + 2, rt * 128:(rt + 1) * 128],
                            lambda w, k: w[k // 8][:, 2 * (k % 8):2 * (k % 8) + 2, :])
                        sl = slice(nch * 512, (nch + 1) * 512)
                        x3 = cw.tile([128, 512], F32, name="x3", tag="x3")
                        nc.vector.scalar_tensor_tensor(
                            out=x3, in0=ps, scalar=1.0 / WSC,
                            in1=x2_my[rt][:, sl], op0=ALU.mult, op1=ALU.add)
                        if not skip_bias:
                            nc.gpsimd.tensor_tensor(out=x3, in0=x3,
                                                    in1=fb2_t[:, sl], op=ALU.add)
                        nc.sync.dma_start(out=out_my[rt * 128:(rt + 1) * 128, sl], in_=x3)

    nc.compile()
    return nc


# ---------------------------------------------------------------------------
# host side
# ---------------------------------------------------------------------------

def _split_fp8(wT):
    """wT [in, out] f32 -> (W0, W1) fp8 e4m3 with W ~ (W0 + W1)/WSC."""
    import ml_dtypes
    f8 = ml_dtypes.float8_e4m3
    ws = (wT * WSC).astype(np.float32)
    w0 = ws.astype(f8)
    w1 = (ws - w0.astype(np.float32)).astype(f8)
    return w0, w1


def _host_prep(inputs):
    import ml_dtypes
    f32 = np.float32
    bf16 = ml_dtypes.bfloat16
    f8 = ml_dtypes.float8_e4m3
    x = np.asarray(inputs["x"], f32)
    y = np.asarray(inputs["y"], f32)
    mask = np.asarray(inputs["mask"])[0, 0]  # [N, N] bool

    causal_ref = np.triu(np.ones((N, N), bool), k=1)
    if np.array_equal(mask, causal_ref):
        mode = "causal"
    elif not mask.any():
        mode = "none"
    else:
        mode = "dense"

    gbs = [("a1_ln1_g", "a1_ln1_b"), ("a1_ln2_g", "a1_ln2_b"),
           ("a2_ln_g", "a2_ln_b"), ("m_ln1_g", "m_ln1_b"), ("m_ln2_g", "m_ln2_b")]
    skip_gb = all(
        np.all(np.asarray(inputs[g]) == 1.0) and np.all(np.asarray(inputs[b]) == 0.0)
        for g, b in gbs)
    skip_bias = all(
        np.all(np.asarray(inputs[b]) == 0.0)
        for b in ("a1_proj_b", "a2_proj_b", "m_fc1_b", "m_fc2_b"))

    wT = lambda k: np.ascontiguousarray(np.asarray(inputs[k], f32).T)
    shared = {}
    for nm, key in (("w_qk", "a1_qk_w"), ("w_v", "a1_v_w"), ("w_pj", "a1_proj_w"),
                    ("w_q2", "a2_q_w"), ("w_kv", "a2_kv_w"), ("w_p2", "a2_proj_w"),
                    ("w_f1", "m_fc1_w"), ("w_f2", "m_fc2_w")):
        w0, w1 = _split_fp8(wT(key))
        shared[nm + "0"] = w0
        shared[nm + "1"] = w1
    if not skip_bias:
        shared["projb"] = np.asarray(inputs["a1_proj_b"], f32)
        shared["proj2b"] = np.asarray(inputs["a2_proj_b"], f32)
        shared["fc1b"] = np.asarray(inputs["m_fc1_b"], f32)
        shared["fc2b"] = np.asarray(inputs["m_fc2_b"], f32)
    if not skip_gb:
        keymap = {"ln1": ("a1_ln1_g", "a1_ln1_b"), "aln2": ("a1_ln2_g", "a1_ln2_b"),
                  "a2ln": ("a2_ln_g", "a2_ln_b"), "mln1": ("m_ln1_g", "m_ln1_b"),
                  "mln2": ("m_ln2_g", "m_ln2_b")}
        for k, (gk, bk) in keymap.items():
            shared["g_" + k] = np.asarray(inputs[gk], f32)
            shared["b_" + k] = np.asarray(inputs[bk], f32)

    in_maps = []
    for core in range(G * CPG):
        g, c = core // CPG, core % CPG
        m = dict(shared)
        m["x_my"] = np.ascontiguousarray(x[g, c::CPG]).astype(bf16)
        m["x_full"] = np.ascontiguousarray(x[g]).astype(bf16)
        m["yT8"] = np.ascontiguousarray(y[g].T).astype(f8)
        if mode == "causal":
            # boundary pair: key (128*jj + kk) vs local row ii in [0, 64):
            # allowed iff 128*jj + kk <= 4*ii + c
            kk = np.arange(128)[:, None, None]
            jj = np.arange(2)[None, :, None]
            ii = np.arange(64)[None, None, :]
            m["bmask"] = np.where(128 * jj + kk <= CPG * ii + c, 0.0,
                                  -1e9).astype(bf16)
        if mode == "dense":
            sub = mask[c::CPG, :]  # [R, N]
            m["maskT"] = np.ascontiguousarray(
                np.where(sub, -1e9, 0.0).astype(f32).T).astype(bf16)
        in_maps.append(m)
    return mode, skip_gb, skip_bias, in_maps


def _assemble(results, dtype):
    out = np.empty((B, N, C), np.float32)
    for core in range(G * CPG):
        g, c = core // CPG, core % CPG
        out[g, c::CPG] = results[core]["out_my"]
    return out.astype(dtype, copy=False)


def get_program(inputs):
    mode, skip_gb, skip_bias, in_maps = _host_prep(inputs)
    key = (mode, skip_gb, skip_bias)
    if key not in _CACHE:
        _CACHE[key] = _build(mode, skip_gb, skip_bias)
    return _CACHE[key], in_maps


def kernel(**inputs):
    from concourse import bass_utils

    nc, in_maps = get_program(inputs)
    res = bass_utils.run_bass_kernel_spmd(nc, in_maps, core_ids=list(range(8)))
    return _assemble(res.results, np.asarray(inputs["x"]).dtype)

